# revision 9
# baseline (speedup 1.0000x reference)
"""Trainium2 Bass kernel for nn_CrossTowerCausalModel.

Data-parallel over graphs: each of the 8 NeuronCores handles 128 graphs
(128*32 = 4096 nodes, 128*64 = 8192 edges). Weights/embeddings replicated.

Device activation layout is "transposed" (layout B): hT[feature, node] with
the 768 feature dim split into 6 chunks of 128 partitions. Weight matrices
[in, out] then serve directly as matmul lhsT (stationary) operands.

Host-side prep (pure index logic + layout, no heavy math):
  * per-graph node permutation so that target node c sits at local slot 0 and
    t at slot 1 -> final gathers h_c / h_t become stride-32 strided copies.
    (c == t graphs are fixed up on device with a predicated copy.)
  * x is passed pre-transposed (feature-major) in bf16 so the projection
    needs no on-device transposes.
  * dense per-graph adjacency (A[t,s] = edge multiplicity), laid out as
    block-diagonal 128x128 tiles covering 4 graphs each -> segment_sum
    becomes small dense matmuls.
  * the quirky first-edge/dist logic of the reference (exact int math).
  * speaker/emotion one-hots (16 rows) fused into the input projection.

All matmuls run with bf16 inputs (full PE rate; fp32 would be 4x slower and
float32r is rejected by the BIR verifier unless every producer rounds to it).
PSUM accumulation and the GNN residual chain stay in fp32, so per-layer
rounding does not compound across layers.
"""

import numpy as np
import ml_dtypes

B = 1024          # graphs
P = 32            # nodes per graph
N = B * P
H = 768
HC = H // 128     # 6 feature chunks
L = 3
DSEM = 1024
NUM_SPK, NUM_EMO = 9, 7
NCORES = 8
BC = B // NCORES          # graphs per core = 128
NCN = BC * P              # nodes per core = 4096
NT = 8                    # node tiles of 512 per core
GPT = 4                   # groups (of 128 nodes) per node tile

BF16 = ml_dtypes.bfloat16

_cache = {}


def _build_program():
    from contextlib import ExitStack

    import concourse.bacc as bacc
    import concourse.mybir as mybir
    import concourse.tile as tile
    from concourse.masks import make_identity

    f32 = mybir.dt.float32
    bf16 = mybir.dt.bfloat16
    AF = mybir.ActivationFunctionType

    nc = bacc.Bacc(
        "TRN2", target_bir_lowering=False, debug=False, num_devices=NCORES
    )

    dram = lambda name, shape, dt: nc.dram_tensor(
        name, shape, dt, kind="ExternalInput"
    ).ap()

    xt = dram("xt", [DSEM, NCN], bf16)
    oh16 = dram("oh16", [16, NCN], bf16)
    embcat = dram("embcat", [16, H], bf16)
    wsem = dram("wsem", [DSEM, H], bf16)
    wself = dram("wself", [L, H, H], bf16)
    wnbr = dram("wnbr", [L, H, H], bf16)
    atb = dram("atb", [NCN // 128, 128, 128], bf16)
    cmask = dram("cmask", [128, BC], mybir.dt.uint8)
    ohd = dram("ohd", [P, BC], bf16)
    demb = dram("demb", [P, H], bf16)
    wexpl = dram("wexpl", [H, H], bf16)
    bexpl = dram("bexpl", [128, HC], f32)
    ext = dram("ext", [H, BC], bf16)
    wp1 = dram("wp1", [6 * H, H], bf16)
    bp1 = dram("bp1", [128, HC], f32)
    wp2 = dram("wp2", [128, HC], bf16)
    bp2 = dram("bp2", [1, 1], f32)
    out_ap = nc.dram_tensor("out", [1, BC], f32, kind="ExternalOutput").ap()

    # [C*128, J] dram AP -> [128, C, J] (partition-major chunked view)
    def chunked(ap, J):
        return ap.rearrange("(c p) j -> c p j", p=128).transpose([1, 0, 2])

    # SBUF tile [128, C*J] -> [128, C, J]
    def sb3(t, J):
        return t[:].rearrange("p (c j) -> p c j", j=J)

    with tile.TileContext(nc) as tc, ExitStack() as ctx:
        erpool = ctx.enter_context(tc.tile_pool(name="er", bufs=1))

        hs = ctx.enter_context(ExitStack())
        hpool = hs.enter_context(tc.tile_pool(name="h", bufs=1))
        hconst = hs.enter_context(tc.tile_pool(name="hconst", bufs=1))

        ident = hconst.tile([128, 128], bf16)
        make_identity(nc, ident)
        atb_t = hconst.tile([128, (NCN // 128) * 128], bf16)
        nc.sync.dma_start(sb3(atb_t, 128), atb.transpose([1, 0, 2]))
        cmask_t = hconst.tile([128, BC], mybir.dt.uint8)
        nc.sync.dma_start(cmask_t[:], cmask[:])

        # persistent transposed activations: hT[jc][nt] is [128, 512] fp32
        hT = [
            [
                hpool.tile(
                    [128, 512], f32, tag=f"h_{jc}_{nt}", name=f"h_{jc}_{nt}"
                )
                for nt in range(NT)
            ]
            for jc in range(HC)
        ]
        # edge_repr^T, 36 chunks of 128 rows: [h_graph_c, h_text_c, h_graph_t,
        # h_text_t, h_dist, z] each HC chunks wide
        erT = erpool.tile([128, 36 * 128], bf16)

        def gather_ct(base_c, base_t):
            # strided gathers of node slot 0 (c) and slot 1 (t) per graph,
            # plus the c==t fixup via predicated copy
            for jc in range(HC):
                for nt in range(NT):
                    src = hT[jc][nt].rearrange("p (b u) -> p b u", u=P)
                    nc.vector.tensor_copy(
                        erT[:, (base_c + jc) * 128 + nt * 16:][:, :16],
                        src[:, :, 0],
                    )
                    nc.vector.tensor_copy(
                        erT[:, (base_t + jc) * 128 + nt * 16:][:, :16],
                        src[:, :, 1],
                    )
                nc.vector.copy_predicated(
                    erT[:, (base_t + jc) * 128:][:, :BC],
                    cmask_t[:],
                    erT[:, (base_c + jc) * 128:][:, :BC],
                )

        # ---------------- phase 1: text projection ----------------
        with ExitStack() as p1:
            xtpool = p1.enter_context(tc.tile_pool(name="xt", bufs=3))
            wsem_pool = p1.enter_context(tc.tile_pool(name="wsem", bufs=1))
            oh_pool = p1.enter_context(tc.tile_pool(name="oh16", bufs=3))
            ps_a = p1.enter_context(tc.tile_pool(name="ps_a", bufs=8, space="PSUM"))

            wsem_t = wsem_pool.tile([128, 8 * H], bf16)
            nc.sync.dma_start(sb3(wsem_t, H), chunked(wsem, H))
            emb_t = wsem_pool.tile([128, H], bf16)
            nc.sync.dma_start(emb_t[:16, :], embcat[:])
            for nt in range(NT):
                oh16_t = oh_pool.tile([128, 512], bf16)
                nc.sync.dma_start(oh16_t[:16, :], oh16[:, nt * 512:][:, :512])
                xt_t = xtpool.tile([128, 8 * 512], bf16)
                nc.sync.dma_start(
                    sb3(xt_t, 512), chunked(xt[:, nt * 512:][:, :512], 512)
                )
                accs = []
                for jc in range(HC):
                    acc = ps_a.tile([128, 512], f32)
                    for kc in range(8):
                        nc.tensor.matmul(
                            acc[:],
                            wsem_t[:, kc * H + jc * 128:][:, :128],
                            xt_t[:, kc * 512:][:, :512],
                            start=(kc == 0),
                            stop=False,
                        )
                    nc.tensor.matmul(
                        acc[:],
                        emb_t[:16, jc * 128:][:, :128],
                        oh16_t[:16, :],
                        start=False,
                        stop=True,
                    )
                    accs.append(acc)
                for jc in range(HC):
                    nc.scalar.activation(hT[jc][nt][:], accs[jc][:], AF.Relu)

            # h_text gathers (chunks 6-11 = h_text_c, 18-23 = h_text_t)
            gather_ct(6, 18)

        # ---------------- phase 2: GNN layers ----------------
        with ExitStack() as p2:
            wpool = p2.enter_context(tc.tile_pool(name="w", bufs=2))
            hbpool = p2.enter_context(tc.tile_pool(name="hb", bufs=2))
            hapool = p2.enter_context(tc.tile_pool(name="ha", bufs=3))
            msgpool = p2.enter_context(tc.tile_pool(name="msg", bufs=2))
            tmppool = p2.enter_context(tc.tile_pool(name="tmp", bufs=3))
            ps_t2 = p2.enter_context(tc.tile_pool(name="ps_t2", bufs=1, space="PSUM"))
            ps_m = p2.enter_context(tc.tile_pool(name="ps_m", bufs=1, space="PSUM"))
            ps_a2 = p2.enter_context(tc.tile_pool(name="ps_a2", bufs=6, space="PSUM"))

            for l in range(L):
                ws_t = wpool.tile([128, HC * H], bf16, tag="ws")
                nc.sync.dma_start(sb3(ws_t, H), chunked(wself[l], H))
                wn_t = wpool.tile([128, HC * H], bf16, tag="wn")
                nc.sync.dma_start(sb3(wn_t, H), chunked(wnbr[l], H))
                for nt in range(NT):
                    # bf16 copy of this node-tile of hT (matmul/transpose input)
                    hb = hbpool.tile([128, HC * 512], bf16)
                    for jc in range(HC):
                        nc.vector.tensor_copy(
                            hb[:, jc * 512:][:, :512], hT[jc][nt][:]
                        )
                    msg_t = msgpool.tile([128, HC * 512], bf16)
                    for g4 in range(GPT):
                        g = nt * GPT + g4
                        ha = hapool.tile([128, H], bf16)
                        for jc in range(HC):
                            pst = ps_t2.tile([128, 128], bf16)
                            nc.tensor.transpose(
                                pst[:],
                                hb[:, jc * 512 + g4 * 128:][:, :128],
                                ident[:],
                            )
                            nc.scalar.activation(
                                ha[:, jc * 128:][:, :128], pst[:], AF.Copy
                            )
                        for jc in range(HC):
                            psm = ps_m.tile([128, 128], f32)
                            nc.tensor.matmul(
                                psm[:],
                                ha[:, jc * 128:][:, :128],
                                atb_t[:, g * 128:][:, :128],
                                start=True,
                                stop=True,
                            )
                            nc.scalar.activation(
                                msg_t[:, jc * 512 + g4 * 128:][:, :128],
                                psm[:],
                                AF.Copy,
                            )
                    accs = []
                    for jc in range(HC):
                        acc = ps_a2.tile([128, 512], f32)
                        for kc in range(HC):
                            nc.tensor.matmul(
                                acc[:],
                                ws_t[:, kc * H + jc * 128:][:, :128],
                                hb[:, kc * 512:][:, :512],
                                start=(kc == 0),
                                stop=False,
                            )
                        for kc in range(HC):
                            nc.tensor.matmul(
                                acc[:],
                                wn_t[:, kc * H + jc * 128:][:, :128],
                                msg_t[:, kc * 512:][:, :512],
                                start=False,
                                stop=(kc == HC - 1),
                            )
                        accs.append(acc)
                    for jc in range(HC):
                        tmp = tmppool.tile([128, 512], f32)
                        nc.scalar.activation(tmp[:], accs[jc][:], AF.Relu)
                        nc.vector.tensor_add(
                            out=hT[jc][nt][:], in0=tmp[:], in1=hT[jc][nt][:]
                        )

        # final h gathers (chunks 0-5 = h_graph_c, 12-17 = h_graph_t)
        gather_ct(0, 12)
        # release hT + GNN constants before the predictor phase
        hs.close()

        # ---------------- phase 3: predictor ----------------
        with ExitStack() as p3:
            ppool = p3.enter_context(tc.tile_pool(name="pred", bufs=1))
            ps_p = p3.enter_context(tc.tile_pool(name="ps_p", bufs=2, space="PSUM"))

            wp1_t = ppool.tile([128, 36 * H], bf16)
            nc.sync.dma_start(sb3(wp1_t, H), chunked(wp1, H))
            ohd_t = ppool.tile([128, BC], bf16)
            nc.sync.dma_start(ohd_t[:P, :], ohd[:])
            demb_t = ppool.tile([128, H], bf16)
            nc.sync.dma_start(demb_t[:P, :], demb[:])
            bexpl_t = ppool.tile([128, HC], f32)
            nc.sync.dma_start(bexpl_t[:], bexpl[:])
            bp1_t = ppool.tile([128, HC], f32)
            nc.sync.dma_start(bp1_t[:], bp1[:])
            wp2_t = ppool.tile([128, HC], bf16)
            nc.sync.dma_start(wp2_t[:], wp2[:])
            bp2_t = ppool.tile([1, 1], f32)
            nc.sync.dma_start(bp2_t[:], bp2[:])
            ext_t = ppool.tile([128, HC * BC], bf16)
            nc.sync.dma_start(sb3(ext_t, BC), chunked(ext, BC))
            wexpl_t = ppool.tile([128, HC * H], bf16)
            nc.sync.dma_start(sb3(wexpl_t, H), chunked(wexpl, H))

            # h_dist (chunks 24-29)
            for jc in range(HC):
                psd = ps_p.tile([128, BC], f32)
                nc.tensor.matmul(
                    psd[:],
                    demb_t[:P, jc * 128:][:, :128],
                    ohd_t[:P, :],
                    start=True,
                    stop=True,
                )
                nc.scalar.activation(erT[:, (24 + jc) * 128:][:, :BC], psd[:], AF.Copy)

            # z_teacher (chunks 30-35)
            for jc in range(HC):
                psz = ps_p.tile([128, BC], f32)
                for kc in range(HC):
                    nc.tensor.matmul(
                        psz[:],
                        wexpl_t[:, kc * H + jc * 128:][:, :128],
                        ext_t[:, kc * BC:][:, :BC],
                        start=(kc == 0),
                        stop=(kc == HC - 1),
                    )
                nc.scalar.activation(
                    erT[:, (30 + jc) * 128:][:, :BC],
                    psz[:],
                    AF.Relu,
                    bias=bexpl_t[:, jc:jc + 1],
                )

            hid_t = ppool.tile([128, HC * BC], bf16)
            for jc in range(HC):
                psp = ps_p.tile([128, BC], f32)
                for kc in range(36):
                    nc.tensor.matmul(
                        psp[:],
                        wp1_t[:, kc * H + jc * 128:][:, :128],
                        erT[:, kc * 128:][:, :128],
                        start=(kc == 0),
                        stop=(kc == 35),
                    )
                nc.scalar.activation(
                    hid_t[:, jc * BC:][:, :BC],
                    psp[:],
                    AF.Relu,
                    bias=bp1_t[:, jc:jc + 1],
                )

            psl = ps_p.tile([128, BC], f32)
            for jc in range(HC):
                nc.tensor.matmul(
                    psl[:1, :],
                    wp2_t[:, jc:jc + 1],
                    hid_t[:, jc * BC:][:, :BC],
                    start=(jc == 0),
                    stop=(jc == HC - 1),
                )
            logit_t = ppool.tile([128, BC], f32)
            nc.vector.tensor_scalar_add(
                out=logit_t[:1, :], in0=psl[:1, :], scalar1=bp2_t[:1, :1]
            )
            nc.sync.dma_start(out_ap[:], logit_t[:1, :])

    nc.compile()
    return nc


def _host_prep(inputs):
    x = np.asarray(inputs["x"], np.float32)
    spk = np.asarray(inputs["speaker_ids"], np.int64)
    emo = np.asarray(inputs["emotion_ids"], np.int64)
    ei = np.asarray(inputs["edge_index"], np.int64)
    tni = np.asarray(inputs["target_node_indices"], np.int64)
    ex = np.asarray(inputs["expl_space_vec"], np.float32)

    E = ei.shape[1]
    edge_src, edge_tgt = ei[0], ei[1]
    c_idx, t_idx = tni[:, 0], tni[:, 1]

    # reference first-edge/dist logic (exact)
    fe = np.full(N, E, np.int64)
    np.minimum.at(fe, edge_src, np.arange(E, dtype=np.int64))

    def first_tgt(q):
        feq = fe[q]
        return np.where(feq < E, edge_tgt[np.minimum(feq, E - 1)], q)

    dist = np.clip(np.abs(first_tgt(c_idx) - first_tgt(t_idx)), 0, P - 1)

    # per-graph permutation: slot 0 = c, slot 1 = t (if distinct)
    prio = np.full((B, P), 2, np.int64)
    prio[np.arange(B), t_idx] = 1
    prio[np.arange(B), c_idx] = 0
    new2old = np.argsort(prio, axis=1, kind="stable")
    old2new = np.argsort(new2old, axis=1)
    perm_global = (np.arange(B)[:, None] * P + new2old).reshape(-1)

    xtb = np.ascontiguousarray(x[perm_global].T.astype(BF16))  # [DSEM, N]
    spk_new = spk[perm_global]
    emo_new = emo[perm_global]

    oh16 = np.zeros((16, N), BF16)
    oh16[spk_new, np.arange(N)] = 1.0
    oh16[NUM_SPK + emo_new, np.arange(N)] = 1.0

    # adjacency in permuted coords, block-diag AT tiles (4 graphs/tile)
    g_e = edge_src // P
    s_new = old2new[g_e, edge_src % P]
    t_new = old2new[g_e, edge_tgt % P]
    A = np.zeros((B, P, P), np.float32)
    np.add.at(A, (g_e, t_new, s_new), 1.0)
    G = B // 4
    atb = np.zeros((G, 128, 128), np.float32)
    Ar = A.reshape(G, 4, P, P)
    for i in range(4):
        atb[:, 32 * i:32 * i + 32, 32 * i:32 * i + 32] = Ar[:, i].transpose(0, 2, 1)
    atb = atb.astype(BF16)

    cmask = np.tile((c_idx == t_idx).astype(np.uint8)[None, :], (128, 1))

    ohd = np.zeros((P, B), BF16)
    ohd[dist, np.arange(B)] = 1.0

    extT = np.ascontiguousarray(ex.T.astype(BF16))

    embcat = np.concatenate(
        [np.asarray(inputs["spk_emb"], np.float32),
         np.asarray(inputs["emo_emb"], np.float32)], 0
    ).astype(BF16)
    rearr = lambda v: np.ascontiguousarray(
        np.asarray(v, np.float32).reshape(HC, 128).T
    )
    b16 = lambda k: np.asarray(inputs[k], np.float32).astype(BF16)

    shared = dict(
        embcat=embcat,
        wsem=b16("W_sem"),
        wself=b16("gnn_w_self"),
        wnbr=b16("gnn_w_nbr"),
        demb=b16("dist_emb"),
        wexpl=b16("W_expl"),
        bexpl=rearr(inputs["b_expl"]),
        wp1=b16("W_p1"),
        bp1=rearr(inputs["b_p1"]),
        wp2=rearr(np.asarray(inputs["W_p2"], np.float32)[:, 0]).astype(BF16),
        bp2=np.asarray(inputs["b_p2"], np.float32).reshape(1, 1),
    )

    in_maps = []
    for i in range(NCORES):
        gs = slice(i * BC, (i + 1) * BC)
        ns = slice(i * NCN, (i + 1) * NCN)
        m = dict(shared)
        m["xt"] = np.ascontiguousarray(xtb[:, ns])
        m["oh16"] = np.ascontiguousarray(oh16[:, ns])
        m["atb"] = np.ascontiguousarray(atb[i * (NCN // 128):(i + 1) * (NCN // 128)])
        m["cmask"] = np.ascontiguousarray(cmask[:, gs])
        m["ohd"] = np.ascontiguousarray(ohd[:, gs])
        m["ext"] = np.ascontiguousarray(extT[:, gs])
        in_maps.append(m)
    return in_maps


def kernel(**inputs):
    in_maps = _host_prep(inputs)
    if "nc" not in _cache:
        _cache["nc"] = _build_program()
    from concourse.bass_utils import run_bass_kernel_spmd

    res = run_bass_kernel_spmd(_cache["nc"], in_maps, list(range(NCORES)))
    out = np.concatenate(
        [res.results[i]["out"].reshape(BC) for i in range(NCORES)]
    )
    return out.astype(np.float32)
